# revision 1
# baseline (speedup 1.0000x reference)
"""Causal self-attention (RoPE, 16 heads, d=64, B=4, T=2048, C=1024) on 8 TRN2 cores.

Sharding: core g = (batch b = g//2, head-group hg = g%2 covering 8 heads).
Data-parallel over B, tensor-parallel over heads.  Each core computes the
partial out-projection (its 8 heads' contribution, no bias); the host sums
the two head-group partials per batch and adds b_out.

Per-core kernel (all matmul operands bf16, fp32 PSUM accumulation), emitted
as one interleaved stream per 512-wide t-window so the PE never starves:

  qkv segment I (dripped into attention window I-1 as PE gap-filler):
    q/k/v = xT.T @ Wqkv (xT pre-transposed on host as the stationary
    operand); RoPE on q,k in natural [t, d] layout on DVE; q,k
    HW-DMA-transposed into [d, t] layout (Sync queue carries ONLY
    transposes so the xbar never flips modes); v stored with 64 ones
    columns appended per head.

  attention window I, per head-pair:
    S^T[s,t] = k^T q with both heads packed in the PE array via
    tile_position row tiling (contraction dim is only 64); causal upper
    blocks skipped; diagonal blocks masked by seeding the PSUM with
    -1e30 above the diagonal via an identity @ negtri matmul before the
    kq matmul accumulates; exp on ACT (scale=1/8, padding mask as
    per-partition bias, no max subtraction -- logits are ~N(0,1));
    AV with [V | ones64] stationary and exp(S^T) streaming accumulates
    attn_out^T[d,t] on psT[0:64] and the denominator replicated on
    psT[64:128], so the softmax division is a wide DVE reciprocal plus
    a GpSimd multiply, landing directly in the out-projection's
    stationary layout (no attention-output transpose);
    "heater" matmuls keep the PE's HAM clock at 2.4 GHz when the
    drip runs dry.
"""

import os
from contextlib import ExitStack

import numpy as np
import ml_dtypes

B, T, C = 4, 2048, 1024
H, D = 16, 64
HG = 8            # heads per core
NCORES = 8
TB = T // 128     # 16 t/s-blocks of 128
CBN = C // 128    # 8 contraction chunks
NP = HG // 2      # 4 head pairs
NI = T // 512     # 4 t-windows of 512
ROPE_BASE = 10000.0

_PROG = None
_LAST_RESULTS = None


def _build_program():
    import concourse.bass as bass
    import concourse.tile as tile
    from concourse import bacc, mybir

    f32 = mybir.dt.float32
    bf = mybir.dt.bfloat16
    EXP = mybir.ActivationFunctionType.Exp

    nc = bacc.Bacc("TRN2", target_bir_lowering=False, debug=False)

    xT = nc.dram_tensor("xT", [C, T], bf, kind="ExternalInput").ap()
    wqkv = nc.dram_tensor("wqkv", [C, 3 * HG * D], bf, kind="ExternalInput").ap()
    wout = nc.dram_tensor("wout", [HG * D, C], bf, kind="ExternalInput").ap()
    cos8 = nc.dram_tensor("cos8", [T, HG * D], bf, kind="ExternalInput").ap()
    sin8 = nc.dram_tensor("sin8", [T, HG * D], bf, kind="ExternalInput").ap()
    padb = nc.dram_tensor("padb", [128, TB], f32, kind="ExternalInput").ap()
    dmask = nc.dram_tensor("dmask", [128, 256], bf, kind="ExternalInput").ap()
    outp = nc.dram_tensor("outp", [T, C], f32, kind="ExternalOutput").ap()

    with tile.TileContext(nc) as tc, ExitStack() as ctx:
        singles = ctx.enter_context(tc.tile_pool(name="singles", bufs=1))

        # ---- global SBUF tensors (input loads on the GpSimd SWDGE queue;
        # the Sync queue is reserved for DMA transposes so the xbar never
        # flips modes) ----
        xt_sb = []
        for cb in range(CBN):
            t_ = singles.tile([128, T], bf, name=f"xt{cb}", tag=f"xt{cb}")
            nc.gpsimd.dma_start(out=t_, in_=xT[cb * 128:(cb + 1) * 128, :])
            xt_sb.append(t_)
        w_sb = []
        for cb in range(CBN):
            t_ = singles.tile([128, 3 * HG * D], bf, name=f"w{cb}", tag=f"w{cb}")
            nc.gpsimd.dma_start(out=t_, in_=wqkv[cb * 128:(cb + 1) * 128, :])
            w_sb.append(t_)
        wo_sb = []
        for c in range(4):
            t_ = singles.tile([128, C], bf, name=f"wo{c}", tag=f"wo{c}")
            nc.gpsimd.dma_start(out=t_, in_=wout[c * 128:(c + 1) * 128, :])
            wo_sb.append(t_)
        cos_sb = singles.tile([128, TB, HG, D], bf, name="cos_sb", tag="cos_sb")
        nc.gpsimd.dma_start(
            out=cos_sb, in_=cos8.rearrange("(tb p) (h d) -> p tb h d", p=128, h=HG))
        sin_sb = singles.tile([128, TB, HG, D], bf, name="sin_sb", tag="sin_sb")
        nc.gpsimd.dma_start(
            out=sin_sb, in_=sin8.rearrange("(tb p) (h d) -> p tb h d", p=128, h=HG))
        padb_sb = singles.tile([128, TB], f32, name="padb_sb", tag="padb_sb")
        nc.gpsimd.dma_start(out=padb_sb, in_=padb)
        dmask_sb = singles.tile([128, 2, 128], bf, name="dmask_sb", tag="dmask_sb")
        nc.gpsimd.dma_start(out=dmask_sb, in_=dmask)

        # q^T/k^T: [within-pair col (head-lo d / head-hi d), s-block, pair, t]
        qT_all = singles.tile([128, TB, NP, 128], bf, name="qT_all", tag="qT_all")
        kT_all = singles.tile([128, TB, NP, 128], bf, name="kT_all", tag="kT_all")
        # v with 64 ones columns per head: the AV matmul then emits the
        # softmax denominator replicated on 64 partitions (rows 64-127),
        # so the reciprocal is a wide DVE op instead of a 1-partition crawl
        vones = singles.tile([128, TB, HG, 128], bf, name="vones", tag="vones")
        nc.vector.memset(vones[:, :, :, D:128], 1.0)

        # qkv + attention interleaved per 512-wide t-window so the PE stream
        # stays dense (HAM stays at 2.4 GHz): attention for window I only
        # needs q/k/v blocks 0..4I+3, which segment I of the qkv loop topped
        # off.  One shared PSUM pool: qkv 2 + sAB 2x2 + psT 2 = 8 banks.
        with tc.tile_pool(name="psum", bufs=2, space="PSUM") as psum, \
             tc.tile_pool(name="rope", bufs=4) as rope_pool, \
             tc.tile_pool(name="qknat", bufs=3) as qk_pool, \
             tc.tile_pool(name="exps", bufs=3) as exp_pool, \
             tc.tile_pool(name="tris", bufs=2) as tri_pool, \
             tc.tile_pool(name="attnT", bufs=2) as aT_pool, \
             tc.tile_pool(name="recips", bufs=2) as rc_pool, \
             tc.tile_pool(name="outsb", bufs=2) as out_pool:
            def qkv_segment(I):
                """Generator emitting segment I's qkv matmuls in half-tensor
                chunks (yield points), so the caller can drip them into the
                attention loop as PE gap-filler while ACT grinds exps."""
                for tb in range(4 * I, 4 * I + 4):
                    for which, base in (("q", 0), ("k", 512), ("v", 1024)):
                        ps = psum.tile([128, HG, D], f32, name=f"ps{which}", tag="qkv")
                        for cb in range(CBN):
                            nc.tensor.matmul(
                                ps, xt_sb[cb][:, tb * 128:(tb + 1) * 128],
                                w_sb[cb][:, base:base + 512],
                                start=(cb == 0), stop=(cb == CBN - 1))
                            if cb == 3:
                                yield
                        if which == "v":
                            nc.vector.tensor_copy(out=vones[:, tb, :, 0:D], in_=ps)
                            yield
                            continue
                        # rope: P_c = qkv*cosF, P_s = qkv*sinF (tables hold the
                        # cos/sin value for BOTH halves of each head), then
                        # lo = P_c.lo - P_s.hi ; hi = P_s.lo + P_c.hi
                        pc = rope_pool.tile([128, HG, D], f32, name="pc", tag="rt")
                        psn = rope_pool.tile([128, HG, D], f32, name="psn", tag="rt")
                        nc.vector.tensor_mul(pc, ps, cos_sb[:, tb])
                        nc.vector.tensor_mul(psn, ps, sin_sb[:, tb])
                        ro = qk_pool.tile([128, HG, D], bf, name="ro", tag="ro")
                        nc.vector.tensor_sub(
                            ro[:, :, 0:32], pc[:, :, 0:32], psn[:, :, 32:64])
                        nc.vector.tensor_add(
                            ro[:, :, 32:64], psn[:, :, 0:32], pc[:, :, 32:64])
                        dst = qT_all if which == "q" else kT_all
                        nc.sync.dma_start_transpose(out=dst[:, tb, :, :], in_=ro)
                        yield

            pending_out = []
            for I in range(NI):
                if I == 0:
                    for _ in qkv_segment(0):
                        pass
                nxt = qkv_segment(I + 1) if I + 1 < NI else None
                n_chunks = 4 * 3 * 3  # yield points per segment
                n_iters = 4 * (4 * I + 4)
                emitted = it = 0

                def drip():
                    nonlocal emitted
                    got = 0
                    if nxt is None:
                        return got
                    # finish the whole segment by ~75% of the window so the
                    # next window's S matmuls never wait on rope/transposes
                    due = (4 * it * n_chunks) // (3 * n_iters)
                    while emitted < due:
                        if next(nxt, "done") == "done":
                            break
                        emitted += 1
                        got += 1
                    return got

                # ---- attention window I ----
                aT_I = aT_pool.tile([128, NP, 512], bf, name="aT_I", tag="aT_I")
                for p in range(NP):
                    psTA = psum.tile([128, 512], f32, name="psTA", tag="avA", bufs=1)
                    psTB = psum.tile([128, 512], f32, name="psTB", tag="avB", bufs=1)

                    def emit_av(j, eAB):
                        off = max(j - 4 * I, 0) * 128
                        for h2, psT in ((0, psTA), (1, psTB)):
                            nc.tensor.matmul(
                                psT[:, off:512],
                                vones[:, j, 2 * p + h2, :],
                                eAB[:, h2, off:512],
                                start=(j == 0), stop=(j == 4 * I + 3))

                    prev = None
                    dry = False
                    for j in range(4 * I + 4):
                        jl = j - 4 * I
                        off = max(jl, 0) * 128
                        sAB = psum.tile([128, 2, 512], f32, name="sAB", tag="sAB", bufs=2)
                        if dry:
                            # "heater" matmul: PE would otherwise sit ~50%
                            # idle behind ACT and HAM-downclock to 1.2 GHz;
                            # burn a throwaway matmul into the bank the next
                            # S matmul overwrites anyway
                            nc.tensor.matmul(
                                sAB[:, 0, :], dmask_sb[:, 0, :], xt_sb[0][:, 0:512],
                                start=True, stop=True, skip_group_check=True)
                        if jl >= 0:
                            # seed the diagonal block with -1e30 above the
                            # diagonal (identity.T @ negtri); the kq matmul
                            # then accumulates on top and exp sees -inf there
                            for h2 in range(2):
                                nc.tensor.matmul(
                                    sAB[:, h2, off:off + 128],
                                    dmask_sb[:, 0, :], dmask_sb[:, 1, :],
                                    start=True, stop=False, skip_group_check=True)
                        HALVES = ((0, slice(0, 64)), (1, slice(64, 128)))
                        if jl >= 0:
                            # keep the row-tiled pair adjacent so the two
                            # heads overlap in the PE array
                            for h2, rows in HALVES:
                                nc.tensor.matmul(
                                    sAB[:, h2, off:off + 128],
                                    kT_all[rows, j, p, :],
                                    qT_all[rows, 4 * I + jl, p, :],
                                    start=False, stop=True,
                                    tile_position=(h2 * 64, 0),
                                    skip_group_check=True)
                            if off + 128 < 512:
                                for h2, rows in HALVES:
                                    nc.tensor.matmul(
                                        sAB[:, h2, off + 128:512],
                                        kT_all[rows, j, p, :],
                                        qT_all[rows, 4 * I + jl + 1:4 * I + 4, p, :],
                                        start=True, stop=True,
                                        tile_position=(h2 * 64, 0))
                        else:
                            for h2, rows in HALVES:
                                nc.tensor.matmul(
                                    sAB[:, h2, :],
                                    kT_all[rows, j, p, :],
                                    qT_all[rows, 4 * I:4 * I + 4, p, :],
                                    start=True, stop=True,
                                    tile_position=(h2 * 64, 0))
                        eAB = exp_pool.tile([128, 2, 512], bf, name="eAB", tag="eAB")
                        bias = padb_sb[:, j:j + 1]
                        nc.scalar.activation(
                            out=eAB[:, :, off:512], in_=sAB[:, :, off:512],
                            func=EXP, bias=bias, scale=0.125)
                        if prev is not None:
                            emit_av(*prev)
                        prev = (j, eAB)
                        it += 1
                        did_out = False
                        if pending_out:
                            pending_out.pop(0)()
                            did_out = True
                        dry = drip() == 0 and not did_out
                    emit_av(*prev)

                    # normalize + evacuate transposed attn straight into the
                    # out-projection's stationary layout.  Two DVE copies free
                    # the PSUM accumulator fast; the 6-cycles-per-element DVE
                    # reciprocal and the normalize multiply then run lazily
                    # off the PE's critical path (multiply on idle GpSimd).
                    for h2, psT in ((0, psTA), (1, psTB)):
                        half = slice(h2 * 64, h2 * 64 + 64)
                        cpn = rc_pool.tile([128, 512], f32, name="cpn", tag="rcn")
                        cpd = rc_pool.tile([128, 512], f32, name="cpd", tag="rcd")
                        nc.vector.tensor_copy(out=cpn[half, :], in_=psT[0:D, :])
                        nc.vector.tensor_copy(out=cpd[half, :], in_=psT[D:128, :])

                        # defer the 3.3us reciprocal + normalize multiply into
                        # the pending queue so they never sit between the PSUM
                        # evacuation copies of consecutive pairs on DVE
                        def make_norm(cpn=cpn, cpd=cpd, aT=aT_I, p=p, half=half):
                            def emit():
                                rc = rc_pool.tile([128, 512], f32, name="rc", tag="rc")
                                nc.vector.reciprocal(rc[half, :], cpd[half, :])
                                nc.gpsimd.tensor_mul(
                                    aT[half, p, :], cpn[half, :], rc[half, :])
                            return emit
                        pending_out.append(make_norm())

                # out-projection units are deferred into the next window's
                # loop as more PE gap-filler
                def make_out_unit(aT, i, il, n):
                    def emit():
                        pso = psum.tile([128, 512], f32, name="pso", tag="sAB", bufs=2)
                        for c in range(4):
                            nc.tensor.matmul(
                                pso,
                                aT[:, c, il * 128:(il + 1) * 128],
                                wo_sb[c][:, n * 512:(n + 1) * 512],
                                start=(c == 0), stop=(c == 3))
                        osb = out_pool.tile([128, 512], f32, name="osb", tag="osb")
                        nc.vector.tensor_copy(out=osb, in_=pso)
                        nc.gpsimd.dma_start(
                            out=outp[i * 128:(i + 1) * 128, n * 512:(n + 1) * 512],
                            in_=osb)
                    return emit
                for il in range(4):
                    for n in range(2):
                        pending_out.append(make_out_unit(aT_I, 4 * I + il, il, n))
                if nxt is not None:
                    for _ in nxt:
                        pass
            for f in pending_out:
                f()

    nc.compile()
    return nc


def _get_program():
    global _PROG
    if _PROG is None:
        _PROG = _build_program()
    return _PROG


def _rope_tables():
    bf16 = ml_dtypes.bfloat16
    inv = 1.0 / (ROPE_BASE ** (np.arange(0, D, 2, dtype=np.float64) / D))
    f = np.arange(T, dtype=np.float64)[:, None] * inv[None, :]  # [T, 32]
    c = np.cos(f)
    s = np.sin(f)
    # per head, both 32-col halves carry the same table value
    cos8 = np.tile(np.concatenate([c, c], axis=1), (1, HG)).astype(bf16)  # [T, 512]
    sin8 = np.tile(np.concatenate([s, s], axis=1), (1, HG)).astype(bf16)
    return cos8, sin8


def kernel(x, attention_mask, W_qkv, W_out, b_out):
    global _LAST_RESULTS
    from concourse.bass_utils import run_bass_kernel_spmd

    nc = _get_program()
    bf16 = ml_dtypes.bfloat16
    x = np.asarray(x, dtype=np.float32)
    attention_mask = np.asarray(attention_mask)
    W_qkv = np.asarray(W_qkv, dtype=np.float32)
    W_out = np.asarray(W_out, dtype=np.float32)
    b_out = np.asarray(b_out, dtype=np.float32)

    cos8, sin8 = _rope_tables()
    iden = np.eye(128, dtype=np.float32)
    negtri = np.where(np.arange(128)[:, None] > np.arange(128)[None, :], -1e30, 0.0)
    dmask = np.concatenate([iden, negtri], axis=1).astype(bf16)

    in_maps = []
    for g in range(NCORES):
        b, hg = g // 2, g % 2
        sl = slice(hg * 512, hg * 512 + 512)
        wq = W_qkv[:, 0 * C:][:, sl]
        wk = W_qkv[:, 1 * C:2 * C][:, sl]
        wv = W_qkv[:, 2 * C:3 * C][:, sl]
        wqkv_g = np.ascontiguousarray(
            np.concatenate([wq, wk, wv], axis=1)).astype(bf16)
        xT_g = np.ascontiguousarray(x[b].T).astype(bf16)
        wout_g = np.ascontiguousarray(W_out[sl, :]).astype(bf16)
        padb_g = np.ascontiguousarray(
            np.where(attention_mask[b] != 0, 0.0, -1e30)
            .astype(np.float32).reshape(TB, 128).T)
        in_maps.append({
            "xT": xT_g, "wqkv": wqkv_g, "wout": wout_g,
            "cos8": cos8, "sin8": sin8, "padb": padb_g, "dmask": dmask,
        })

    res = run_bass_kernel_spmd(nc, in_maps, list(range(NCORES)))
    _LAST_RESULTS = res
    out = np.empty((B, T, C), dtype=np.float32)
    for b in range(B):
        out[b] = res.results[2 * b]["outp"] + res.results[2 * b + 1]["outp"] + b_out
    return out



# revision 18
# speedup vs baseline: 15098.0207x; 15098.0207x over previous
"""Causal self-attention (RoPE, 16 heads, d=64, B=4, T=2048, C=1024) on 8 TRN2 cores.

Sharding: core g = (batch b = g//2, head-group hg = g%2 covering 8 heads).
Data-parallel over B, tensor-parallel over heads.  Each core computes the
partial out-projection (its 8 heads' contribution, no bias); the host sums
the two head-group partials per batch and adds b_out.

Per-core kernel (all matmul operands bf16, fp32 PSUM accumulation), emitted
as one interleaved stream per 512-wide t-window so the PE never starves:

  startup: inputs split across 4 DGE queues (xt on gpsimd+scalar, wqkv on
    vector, tables on tensor) so the first qkv unit is compute- not
    DMA-gated; cos/sin tables loaded compact [T,64] and broadcast across
    heads with stride-0 APs; wout + out-proj stores on the gpsimd queue.

  qkv units dripped into the attention windows as PE gap-filler: window I
    drips its own v units (needed by its AVs) plus q/k of window I+1
    (window 0 runs behind q/k-only preamble).  RoPE runs in bf16 on DVE
    (PSUM evacuated via one cast-copy, then 2-byte fast-mode mul/sub/add);
    q,k HW-DMA-transposed into [d, t] layout (Sync queue carries ONLY
    transposes so the xbar never flips modes); v stored with 64 ones
    columns appended per head.

  attention window I, per head-pair:
    S^T[s,t] = k^T q with both heads packed in the PE array via
    tile_position row tiling (contraction dim is only 64); causal upper
    blocks skipped; ONE matmul per (head, s-block) covering [off:512] --
    the diagonal block is masked after exp by a 0/1-triangle multiply on
    DVE (cheaper than PSUM seed matmuls); exp on ACT (scale=1/8, padding
    mask as per-partition bias, no max subtraction -- logits are ~N(0,1));
    AV with [V | ones64] stationary and exp(S^T) streaming accumulates
    attn_out^T[d,t] on psT[0:64] and the denominator replicated on
    psT[64:128]; the softmax division is a reciprocal_approx_fast on DVE
    straight from PSUM plus a GpSimd multiply, landing directly in the
    out-projection's stationary layout (no attention-output transpose);
    "heater" matmuls keep the PE's HAM clock at 2.4 GHz when the
    drip runs dry.
"""

import os
from contextlib import ExitStack

import numpy as np
import ml_dtypes

B, T, C = 4, 2048, 1024
H, D = 16, 64
HG = 8            # heads per core
NCORES = 8
TB = T // 128     # 16 t/s-blocks of 128
CBN = C // 128    # 8 contraction chunks
NP = HG // 2      # 4 head pairs
NI = T // 512     # 4 t-windows of 512
ROPE_BASE = 10000.0

_PROG = None
_LAST_RESULTS = None


def _build_program():
    import concourse.bass as bass
    import concourse.tile as tile
    from concourse import bacc, mybir

    f32 = mybir.dt.float32
    bf = mybir.dt.bfloat16
    EXP = mybir.ActivationFunctionType.Exp

    nc = bacc.Bacc("TRN2", target_bir_lowering=False, debug=False)

    xT = nc.dram_tensor("xT", [C, T], bf, kind="ExternalInput").ap()
    wqkv = nc.dram_tensor("wqkv", [C, 3 * HG * D], bf, kind="ExternalInput").ap()
    wout = nc.dram_tensor("wout", [HG * D, C], bf, kind="ExternalInput").ap()
    cosc = nc.dram_tensor("cosc", [T, D], bf, kind="ExternalInput").ap()
    sinc = nc.dram_tensor("sinc", [T, D], bf, kind="ExternalInput").ap()
    padb = nc.dram_tensor("padb", [128, TB], f32, kind="ExternalInput").ap()
    dmask = nc.dram_tensor("dmask", [128, 256], bf, kind="ExternalInput").ap()
    outp = nc.dram_tensor("outp", [T, C], f32, kind="ExternalOutput").ap()

    with tile.TileContext(nc) as tc, ExitStack() as ctx:
        singles = ctx.enter_context(tc.tile_pool(name="singles", bufs=1))

        # ---- global SBUF tensors.  The preamble (q/k units for window 0)
        # only needs xt columns 0:512 (1MB of the 4MB) plus the q/k halves
        # of wqkv (2MB): that 3MB critical set is spread over all three DGE
        # queues (gpsimd SWDGE ~300GB/s, scalar/sync HWDGE ~150GB/s each).
        # Later xt t-quarters are JIT-loaded inside each window's feed; the
        # v-half of wqkv, wout, and the tables follow the critical set.
        # The Sync queue reverts to transposes-only once its two critical
        # descriptors retire. ----
        xt_sb = [None] * CBN
        w_sb = [None] * CBN
        for cb in range(CBN):
            xt_sb[cb] = singles.tile([128, T], bf, name=f"xt{cb}", tag=f"xt{cb}")
            w_sb[cb] = singles.tile([128, 3 * HG * D], bf, name=f"w{cb}", tag=f"w{cb}")
        padb_sb = singles.tile([128, TB], f32, name="padb_sb", tag="padb_sb")
        dmask_sb = singles.tile([128, 2, 128], bf, name="dmask_sb", tag="dmask_sb")
        cos_sb = singles.tile([128, TB, D], bf, name="cos_sb", tag="cos_sb")
        sin_sb = singles.tile([128, TB, D], bf, name="sin_sb", tag="sin_sb")

        padb_sb2 = None  # (placeholder removed)
        nc.scalar.dma_start(out=padb_sb, in_=padb)
        nc.scalar.dma_start(out=dmask_sb, in_=dmask)
        nc.scalar.dma_start(
            out=cos_sb, in_=cosc.rearrange("(tb p) d -> p tb d", p=128))
        nc.scalar.dma_start(
            out=sin_sb, in_=sinc.rearrange("(tb p) d -> p tb d", p=128))
        for cb in range(CBN):
            xq, wq = (nc.gpsimd, nc.scalar) if cb % 2 == 0 else (nc.scalar, nc.gpsimd)
            xq.dma_start(out=xt_sb[cb], in_=xT[cb * 128:(cb + 1) * 128, :])
            wq.dma_start(out=w_sb[cb], in_=wqkv[cb * 128:(cb + 1) * 128, :])
        wo_sb = []
        for c in range(4):
            t_ = singles.tile([128, C], bf, name=f"wo{c}", tag=f"wo{c}")
            nc.gpsimd.dma_start(out=t_, in_=wout[c * 128:(c + 1) * 128, :])
            wo_sb.append(t_)

        # q^T/k^T: [within-pair col (head-lo d / head-hi d), s-block, pair, t]
        qT_all = singles.tile([128, TB, NP, 128], bf, name="qT_all", tag="qT_all")
        kT_all = singles.tile([128, TB, NP, 128], bf, name="kT_all", tag="kT_all")
        # v with 64 ones columns per head: the AV matmul then emits the
        # softmax denominator replicated on 64 partitions (rows 64-127)
        vones = singles.tile([128, TB, HG, 128], bf, name="vones", tag="vones")
        nc.gpsimd.memset(vones[:, :, :, D:128], 1.0)

        with tc.tile_pool(name="psum", bufs=2, space="PSUM") as psum, \
             tc.tile_pool(name="rope", bufs=6) as rope_pool, \
             tc.tile_pool(name="qknat", bufs=3) as qk_pool, \
             tc.tile_pool(name="exps", bufs=3) as exp_pool, \
             tc.tile_pool(name="attnT", bufs=2) as aT_pool, \
             tc.tile_pool(name="recips", bufs=4) as rc_pool, \
             tc.tile_pool(name="outsb", bufs=2) as out_pool:

            def qkv_unit(tb, which):
                """Generator emitting one q/k/v projection unit in
                half-tensor chunks (yield points) so the caller can drip
                it into the attention loop as PE gap-filler."""
                base = {"q": 0, "k": 512, "v": 1024}[which]
                ps = psum.tile([128, HG, D], f32, name=f"ps{which}", tag="qkv")
                for cb in range(CBN):
                    nc.tensor.matmul(
                        ps, xt_sb[cb][:, tb * 128:(tb + 1) * 128],
                        w_sb[cb][:, base:base + 512],
                        start=(cb == 0), stop=(cb == CBN - 1))
                    if cb == 3:
                        yield
                if which == "v":
                    nc.vector.tensor_copy(out=vones[:, tb, :, 0:D], in_=ps)
                    yield
                    return
                # rope in bf16: one cast-copy evacuates the PSUM, then the
                # 2-byte fast-mode muls/adds run at 2-4x DVE rate.
                # P_c = qkv*cosF, P_s = qkv*sinF (tables hold the cos/sin
                # value for BOTH halves of each head, broadcast across the
                # 8 heads via a stride-0 AP), then
                # lo = P_c.lo - P_s.hi ; hi = P_s.lo + P_c.hi
                qn = rope_pool.tile([128, HG, D], bf, name="qn", tag="rt")
                nc.vector.tensor_copy(out=qn, in_=ps)
                cos_bc = cos_sb[:, tb].unsqueeze(1).broadcast_to((128, HG, D))
                sin_bc = sin_sb[:, tb].unsqueeze(1).broadcast_to((128, HG, D))
                pc = rope_pool.tile([128, HG, D], bf, name="pc", tag="rt")
                psn = rope_pool.tile([128, HG, D], bf, name="psn", tag="rt")
                nc.vector.tensor_mul(pc, qn, cos_bc)
                nc.vector.tensor_mul(psn, qn, sin_bc)
                ro = qk_pool.tile([128, HG, D], bf, name="ro", tag="ro")
                nc.vector.tensor_sub(
                    ro[:, :, 0:32], pc[:, :, 0:32], psn[:, :, 32:64])
                nc.vector.tensor_add(
                    ro[:, :, 32:64], psn[:, :, 0:32], pc[:, :, 32:64])
                dst = qT_all if which == "q" else kT_all
                nc.sync.dma_start_transpose(out=dst[:, tb, :, :], in_=ro)
                yield

            def feed(I):
                """Drip feed for window I: first the units this window's own
                S/AVs consume (window 0 also picks up its trailing k units),
                then q/k of window I+1."""
                for tb in range(4 * I, 4 * I + 4):
                    yield from qkv_unit(tb, "v")
                if I + 1 < NI:
                    for tb in range(4 * (I + 1), 4 * (I + 1) + 4):
                        yield from qkv_unit(tb, "q")
                        yield from qkv_unit(tb, "k")

            # preamble: the minimum window 0 needs to start (all q, k0, k1);
            # k2/k3 drip in at the front of window 0's feed
            for tb in range(4):
                for _ in qkv_unit(tb, "q"):
                    pass
            for tb in range(4):
                for _ in qkv_unit(tb, "k"):
                    pass

            pending_out = []
            for I in range(NI):
                nxt = feed(I)
                n_a = 8                      # yield points in the eager part
                rate = 2
                n_qk = 16 if I + 1 < NI else 0
                n_iters = 4 * (4 * I + 4)
                qk_span = max(1, (3 * n_iters) // 4 - 4)
                emitted = it = 0

                def drip():
                    nonlocal emitted
                    got = 0
                    # eager units land in the first iterations (this window's
                    # S/AVs need them); q/k spread until ~75% of the window
                    due = min(rate * (it + 1), n_a)
                    if it >= 4:
                        due = n_a + min(n_qk, (((it - 4) * n_qk) // qk_span))
                    while emitted < due:
                        if next(nxt, "done") == "done":
                            break
                        emitted += 1
                        got += 1
                    return got

                # ---- attention window I ----
                aT_I = aT_pool.tile([128, NP, 512], bf, name="aT_I", tag="aT_I")
                for p in range(NP):
                    psTA = psum.tile([128, 512], f32, name="psTA", tag="avA", bufs=1)
                    psTB = psum.tile([128, 512], f32, name="psTB", tag="avB", bufs=1)

                    def emit_av(j, eAB):
                        off = max(j - 4 * I, 0) * 128
                        for h2, psT in ((0, psTA), (1, psTB)):
                            nc.tensor.matmul(
                                psT[:, off:512],
                                vones[:, j, 2 * p + h2, :],
                                eAB[:, h2, off:512],
                                start=(j == 0), stop=(j == 4 * I + 3))

                    prev = None
                    dry = False
                    for j in range(4 * I + 4):
                        jl = j - 4 * I
                        lo = max(jl, 0)
                        off = lo * 128
                        sAB = psum.tile([128, 2, 512], f32, name="sAB", tag="sAB", bufs=2)
                        if dry:
                            # "heater" matmul: PE would otherwise sit ~50%
                            # idle behind ACT and HAM-downclock to 1.2 GHz;
                            # burn a throwaway matmul into the bank the next
                            # S matmul overwrites anyway
                            nc.tensor.matmul(
                                sAB[:, 0, :], dmask_sb[:, 0, :], xt_sb[0][:, 0:512],
                                start=True, stop=True, skip_group_check=True)
                        # S^T for s-block j against all remaining t-blocks in
                        # one matmul per head (the diagonal block's upper
                        # triangle is zeroed after exp, not PSUM-seeded)
                        HALVES = ((0, slice(0, 64)), (1, slice(64, 128)))
                        for h2, rows in HALVES:
                            nc.tensor.matmul(
                                sAB[:, h2, off:512],
                                kT_all[rows, j, p, :],
                                qT_all[rows, 4 * I + lo:4 * I + 4, p, :],
                                start=True, stop=True,
                                tile_position=(h2 * 64, 0),
                                skip_group_check=True)
                        eAB = exp_pool.tile([128, 2, 512], bf, name="eAB", tag="eAB")
                        bias = padb_sb[:, j:j + 1]
                        nc.scalar.activation(
                            out=eAB[:, :, off:512], in_=sAB[:, :, off:512],
                            func=EXP, bias=bias, scale=0.125)
                        if jl >= 0:
                            # zero the diagonal block above the diagonal
                            # (both heads in one 2-byte fast-mode multiply)
                            nc.vector.tensor_mul(
                                eAB[:, :, off:off + 128],
                                eAB[:, :, off:off + 128], dmask_sb)
                        if prev is not None:
                            emit_av(*prev)
                        prev = (j, eAB)
                        it += 1
                        did_out = False
                        if pending_out:
                            pending_out.pop(0)()
                            did_out = True
                        dry = drip() == 0 and not did_out
                    emit_av(*prev)

                    # normalize + evacuate transposed attn straight into the
                    # out-projection's stationary layout: DVE copies free the
                    # PSUM accumulators fast, then ONE full-width
                    # reciprocal_approx_fast covers both heads (~0.7us vs
                    # 2x3.3us for the exact reciprocal; the custom DVE op
                    # requires base partition 0, hence the cpd staging); the
                    # normalize multiplies run lazily on idle GpSimd off the
                    # PE's critical path.
                    cpd = rc_pool.tile([128, 512], f32, name="cpd", tag="rcd")
                    cpns = []
                    for h2, psT in ((0, psTA), (1, psTB)):
                        half = slice(h2 * 64, h2 * 64 + 64)
                        cpn = rc_pool.tile([128, 512], f32, name="cpn", tag="rcn")
                        nc.vector.tensor_copy(out=cpn[half, :], in_=psT[0:D, :])
                        nc.vector.tensor_copy(out=cpd[half, :], in_=psT[D:128, :])
                        cpns.append((half, cpn))
                    rc = rc_pool.tile([128, 512], f32, name="rc", tag="rc")
                    nc.vector.reciprocal_approx_fast(rc, cpd)
                    for half, cpn in cpns:
                        nc.gpsimd.tensor_mul(
                            aT_I[half, p, :], cpn[half, :], rc[half, :])

                # out-projection units are deferred into the next window's
                # loop as more PE gap-filler
                def make_out_unit(aT, i, il, n):
                    def emit():
                        pso = psum.tile([128, 512], f32, name="pso", tag="sAB", bufs=2)
                        for c in range(4):
                            nc.tensor.matmul(
                                pso,
                                aT[:, c, il * 128:(il + 1) * 128],
                                wo_sb[c][:, n * 512:(n + 1) * 512],
                                start=(c == 0), stop=(c == 3))
                        dst = outp[i * 128:(i + 1) * 128, n * 512:(n + 1) * 512]
                        osb = out_pool.tile([128, 512], f32, name="osb", tag="osb")
                        nc.vector.tensor_copy(out=osb, in_=pso)
                        nc.gpsimd.dma_start(out=dst, in_=osb)
                    return emit
                for il in range(4):
                    for n in range(2):
                        pending_out.append(make_out_unit(aT_I, 4 * I + il, il, n))
                for _ in nxt:
                    pass
            for f in pending_out:
                f()

    nc.compile()
    return nc


def _get_program():
    global _PROG
    if _PROG is None:
        _PROG = _build_program()
    return _PROG


def _rope_tables():
    bf16 = ml_dtypes.bfloat16
    inv = 1.0 / (ROPE_BASE ** (np.arange(0, D, 2, dtype=np.float64) / D))
    f = np.arange(T, dtype=np.float64)[:, None] * inv[None, :]  # [T, 32]
    c = np.cos(f)
    s = np.sin(f)
    # both 32-col halves carry the same table value
    cosc = np.concatenate([c, c], axis=1).astype(bf16)  # [T, 64]
    sinc = np.concatenate([s, s], axis=1).astype(bf16)
    return cosc, sinc


def kernel(x, attention_mask, W_qkv, W_out, b_out):
    global _LAST_RESULTS
    from concourse.bass_utils import run_bass_kernel_spmd

    nc = _get_program()
    bf16 = ml_dtypes.bfloat16
    x = np.asarray(x, dtype=np.float32)
    attention_mask = np.asarray(attention_mask)
    W_qkv = np.asarray(W_qkv, dtype=np.float32)
    W_out = np.asarray(W_out, dtype=np.float32)
    b_out = np.asarray(b_out, dtype=np.float32)

    cosc, sinc = _rope_tables()
    tri01 = (np.arange(128)[:, None] <= np.arange(128)[None, :]).astype(np.float32)
    dmask = np.concatenate([tri01, tri01], axis=1).astype(bf16)  # [128, 2*128]

    in_maps = []
    for g in range(NCORES):
        b, hg = g // 2, g % 2
        sl = slice(hg * 512, hg * 512 + 512)
        wq = W_qkv[:, 0 * C:][:, sl]
        wk = W_qkv[:, 1 * C:2 * C][:, sl]
        wv = W_qkv[:, 2 * C:3 * C][:, sl]
        wqkv_g = np.ascontiguousarray(
            np.concatenate([wq, wk, wv], axis=1)).astype(bf16)
        xT_g = np.ascontiguousarray(x[b].T).astype(bf16)
        wout_g = np.ascontiguousarray(W_out[sl, :]).astype(bf16)
        padb_g = np.ascontiguousarray(
            np.where(attention_mask[b] != 0, 0.0, -1e30)
            .astype(np.float32).reshape(TB, 128).T)
        in_maps.append({
            "xT": xT_g, "wqkv": wqkv_g, "wout": wout_g,
            "cosc": cosc, "sinc": sinc, "padb": padb_g, "dmask": dmask,
        })

    res = run_bass_kernel_spmd(nc, in_maps, list(range(NCORES)))
    _LAST_RESULTS = res
    out = np.empty((B, T, C), dtype=np.float32)
    for b in range(B):
        out[b] = res.results[2 * b]["outp"] + res.results[2 * b + 1]["outp"] + b_out
    return out


# revision 19
# speedup vs baseline: 15753.5263x; 1.0434x over previous
"""Causal self-attention (RoPE, 16 heads, d=64, B=4, T=2048, C=1024) on 8 TRN2 cores.

Sharding: core g = (batch b = g//2, head-group hg = g%2 covering 8 heads).
Data-parallel over B, tensor-parallel over heads.  Each core computes the
partial out-projection (its 8 heads' contribution, no bias); the host sums
the two head-group partials per batch and adds b_out.

Per-core kernel (all matmul operands bf16, fp32 PSUM accumulation), emitted
as one interleaved stream per 512-wide t-window so the PE never starves:

  startup: inputs split across 4 DGE queues (xt on gpsimd+scalar, wqkv on
    vector, tables on tensor) so the first qkv unit is compute- not
    DMA-gated; cos/sin tables loaded compact [T,64] and broadcast across
    heads with stride-0 APs; wout + out-proj stores on the gpsimd queue.

  qkv units dripped into the attention windows as PE gap-filler: window I
    drips its own v units (needed by its AVs) plus q/k of window I+1
    (window 0 runs behind q/k-only preamble).  RoPE runs in bf16 on DVE
    (PSUM evacuated via one cast-copy, then 2-byte fast-mode mul/sub/add);
    q,k HW-DMA-transposed into [d, t] layout (Sync queue carries ONLY
    transposes so the xbar never flips modes); v stored with 64 ones
    columns appended per head.

  attention window I, per head-pair:
    S^T[s,t] = k^T q with both heads packed in the PE array via
    tile_position row tiling (contraction dim is only 64); causal upper
    blocks skipped; ONE matmul per (head, s-block) covering [off:512] --
    the diagonal block is masked after exp by a 0/1-triangle multiply on
    DVE (cheaper than PSUM seed matmuls); exp on ACT (scale=1/8, padding
    mask as per-partition bias, no max subtraction -- logits are ~N(0,1));
    AV with [V | ones64] stationary and exp(S^T) streaming accumulates
    attn_out^T[d,t] on psT[0:64] and the denominator replicated on
    psT[64:128]; the softmax division is a reciprocal_approx_fast on DVE
    straight from PSUM plus a GpSimd multiply, landing directly in the
    out-projection's stationary layout (no attention-output transpose);
    "heater" matmuls keep the PE's HAM clock at 2.4 GHz when the
    drip runs dry.
"""

import os
from contextlib import ExitStack

import numpy as np
import ml_dtypes

B, T, C = 4, 2048, 1024
H, D = 16, 64
HG = 8            # heads per core
NCORES = 8
TB = T // 128     # 16 t/s-blocks of 128
CBN = C // 128    # 8 contraction chunks
NP = HG // 2      # 4 head pairs
NI = T // 512     # 4 t-windows of 512
ROPE_BASE = 10000.0

_PROG = None
_LAST_RESULTS = None


def _build_program():
    import concourse.bass as bass
    import concourse.tile as tile
    from concourse import bacc, mybir

    f32 = mybir.dt.float32
    bf = mybir.dt.bfloat16
    EXP = mybir.ActivationFunctionType.Exp

    nc = bacc.Bacc("TRN2", target_bir_lowering=False, debug=False)

    xT = nc.dram_tensor("xT", [C, T], bf, kind="ExternalInput").ap()
    wqkv = nc.dram_tensor("wqkv", [C, 3 * HG * D], bf, kind="ExternalInput").ap()
    wout = nc.dram_tensor("wout", [HG * D, C], bf, kind="ExternalInput").ap()
    cosc = nc.dram_tensor("cosc", [T, D], bf, kind="ExternalInput").ap()
    sinc = nc.dram_tensor("sinc", [T, D], bf, kind="ExternalInput").ap()
    padb = nc.dram_tensor("padb", [128, TB], f32, kind="ExternalInput").ap()
    dmask = nc.dram_tensor("dmask", [128, 256], bf, kind="ExternalInput").ap()
    outp = nc.dram_tensor("outp", [T, C], f32, kind="ExternalOutput").ap()

    with tile.TileContext(nc) as tc, ExitStack() as ctx:
        singles = ctx.enter_context(tc.tile_pool(name="singles", bufs=1))

        # ---- global SBUF tensors.  The preamble (q/k units for window 0)
        # only needs xt columns 0:512 (1MB of the 4MB) plus the q/k halves
        # of wqkv (2MB): that 3MB critical set is spread over all three DGE
        # queues (gpsimd SWDGE ~300GB/s, scalar/sync HWDGE ~150GB/s each).
        # Later xt t-quarters are JIT-loaded inside each window's feed; the
        # v-half of wqkv, wout, and the tables follow the critical set.
        # The Sync queue reverts to transposes-only once its two critical
        # descriptors retire. ----
        xt_sb = [None] * CBN
        w_sb = [None] * CBN
        for cb in range(CBN):
            xt_sb[cb] = singles.tile([128, T], bf, name=f"xt{cb}", tag=f"xt{cb}")
            w_sb[cb] = singles.tile([128, 3 * HG * D], bf, name=f"w{cb}", tag=f"w{cb}")
        padb_sb = singles.tile([128, TB], f32, name="padb_sb", tag="padb_sb")
        dmask_sb = singles.tile([128, 2, 128], bf, name="dmask_sb", tag="dmask_sb")
        cos_sb = singles.tile([128, TB, D], bf, name="cos_sb", tag="cos_sb")
        sin_sb = singles.tile([128, TB, D], bf, name="sin_sb", tag="sin_sb")

        padb_sb2 = None  # (placeholder removed)
        nc.scalar.dma_start(out=padb_sb, in_=padb)
        nc.scalar.dma_start(out=dmask_sb, in_=dmask)
        nc.scalar.dma_start(
            out=cos_sb, in_=cosc.rearrange("(tb p) d -> p tb d", p=128))
        nc.scalar.dma_start(
            out=sin_sb, in_=sinc.rearrange("(tb p) d -> p tb d", p=128))
        for cb in range(CBN):
            xq, wq = (nc.gpsimd, nc.scalar) if cb % 2 == 0 else (nc.scalar, nc.gpsimd)
            xq.dma_start(out=xt_sb[cb], in_=xT[cb * 128:(cb + 1) * 128, :])
            wq.dma_start(out=w_sb[cb], in_=wqkv[cb * 128:(cb + 1) * 128, :])
        wo_sb = []
        for c in range(4):
            t_ = singles.tile([128, C], bf, name=f"wo{c}", tag=f"wo{c}")
            nc.gpsimd.dma_start(out=t_, in_=wout[c * 128:(c + 1) * 128, :])
            wo_sb.append(t_)

        # q^T/k^T: [within-pair col (head-lo d / head-hi d), s-block, pair, t]
        qT_all = singles.tile([128, TB, NP, 128], bf, name="qT_all", tag="qT_all")
        kT_all = singles.tile([128, TB, NP, 128], bf, name="kT_all", tag="kT_all")
        # v with 64 ones columns per head: the AV matmul then emits the
        # softmax denominator replicated on 64 partitions (rows 64-127)
        vones = singles.tile([128, TB, HG, 128], bf, name="vones", tag="vones")
        nc.gpsimd.memset(vones[:, :, :, D:128], 1.0)

        with tc.tile_pool(name="psum", bufs=2, space="PSUM") as psum, \
             tc.tile_pool(name="rope", bufs=6) as rope_pool, \
             tc.tile_pool(name="qknat", bufs=3) as qk_pool, \
             tc.tile_pool(name="exps", bufs=3) as exp_pool, \
             tc.tile_pool(name="attnT", bufs=2) as aT_pool, \
             tc.tile_pool(name="recips", bufs=4) as rc_pool, \
             tc.tile_pool(name="outsb", bufs=2) as out_pool:

            def qkv_unit(tb, which):
                """Generator emitting one q/k/v projection unit in
                half-tensor chunks (yield points) so the caller can drip
                it into the attention loop as PE gap-filler."""
                base = {"q": 0, "k": 512, "v": 1024}[which]
                ps = psum.tile([128, HG, D], f32, name=f"ps{which}", tag="qkv")
                for cb in range(CBN):
                    nc.tensor.matmul(
                        ps, xt_sb[cb][:, tb * 128:(tb + 1) * 128],
                        w_sb[cb][:, base:base + 512],
                        start=(cb == 0), stop=(cb == CBN - 1))
                    if cb == 3:
                        yield
                if which == "v":
                    nc.vector.tensor_copy(out=vones[:, tb, :, 0:D], in_=ps)
                    yield
                    return
                # rope in bf16: one cast-copy evacuates the PSUM, then the
                # 2-byte fast-mode muls/adds run at 2-4x DVE rate.
                # P_c = qkv*cosF, P_s = qkv*sinF (tables hold the cos/sin
                # value for BOTH halves of each head, broadcast across the
                # 8 heads via a stride-0 AP), then
                # lo = P_c.lo - P_s.hi ; hi = P_s.lo + P_c.hi
                qn = rope_pool.tile([128, HG, D], bf, name="qn", tag="rt")
                nc.vector.tensor_copy(out=qn, in_=ps)
                cos_bc = cos_sb[:, tb].unsqueeze(1).broadcast_to((128, HG, D))
                sin_bc = sin_sb[:, tb].unsqueeze(1).broadcast_to((128, HG, D))
                pc = rope_pool.tile([128, HG, D], bf, name="pc", tag="rt")
                psn = rope_pool.tile([128, HG, D], bf, name="psn", tag="rt")
                nc.vector.tensor_mul(pc, qn, cos_bc)
                nc.vector.tensor_mul(psn, qn, sin_bc)
                ro = qk_pool.tile([128, HG, D], bf, name="ro", tag="ro")
                nc.vector.tensor_sub(
                    ro[:, :, 0:32], pc[:, :, 0:32], psn[:, :, 32:64])
                nc.vector.tensor_add(
                    ro[:, :, 32:64], psn[:, :, 0:32], pc[:, :, 32:64])
                dst = qT_all if which == "q" else kT_all
                nc.sync.dma_start_transpose(out=dst[:, tb, :, :], in_=ro)
                yield

            def feed(I):
                """Drip feed for window I: first the units this window's own
                S/AVs consume (window 0 also picks up its trailing k units),
                then q/k of window I+1."""
                for tb in range(4 * I, 4 * I + 4):
                    yield from qkv_unit(tb, "v")
                if I + 1 < NI:
                    for tb in range(4 * (I + 1), 4 * (I + 1) + 4):
                        yield from qkv_unit(tb, "q")
                        yield from qkv_unit(tb, "k")

            # preamble: the minimum window 0 needs to start (all q, k0, k1);
            # k2/k3 drip in at the front of window 0's feed
            for tb in range(4):
                for _ in qkv_unit(tb, "q"):
                    pass
            for tb in range(4):
                for _ in qkv_unit(tb, "k"):
                    pass

            pending_out = []
            for I in range(NI):
                nxt = feed(I)
                n_a = 8                      # yield points in the eager part
                rate = 2
                n_qk = 16 if I + 1 < NI else 0
                n_iters = 4 * (4 * I + 4)
                qk_span = max(1, (3 * n_iters) // 4 - 4)
                emitted = it = 0

                def drip():
                    nonlocal emitted
                    got = 0
                    # eager units land in the first iterations (this window's
                    # S/AVs need them); q/k spread until ~75% of the window
                    due = min(rate * (it + 1), n_a)
                    if it >= 4:
                        due = n_a + min(n_qk, (((it - 4) * n_qk) // qk_span))
                    while emitted < due:
                        if next(nxt, "done") == "done":
                            break
                        emitted += 1
                        got += 1
                    return got

                # ---- attention window I ----
                aT_I = aT_pool.tile([128, NP, 512], bf, name="aT_I", tag="aT_I")
                for p in range(NP):
                    psTA = psum.tile([128, 512], f32, name="psTA", tag="avA", bufs=1)
                    psTB = psum.tile([128, 512], f32, name="psTB", tag="avB", bufs=1)

                    def emit_av(j, eAB):
                        off = max(j - 4 * I, 0) * 128
                        for h2, psT in ((0, psTA), (1, psTB)):
                            nc.tensor.matmul(
                                psT[:, off:512],
                                vones[:, j, 2 * p + h2, :],
                                eAB[:, h2, off:512],
                                start=(j == 0), stop=(j == 4 * I + 3))

                    prev = None
                    dry = False
                    for j in range(4 * I + 4):
                        jl = j - 4 * I
                        lo = max(jl, 0)
                        off = lo * 128
                        sAB = psum.tile([128, 2, 512], f32, name="sAB", tag="sAB", bufs=2)
                        if dry:
                            # "heater" matmul: PE would otherwise sit ~50%
                            # idle behind ACT and HAM-downclock to 1.2 GHz;
                            # burn a throwaway matmul into the bank the next
                            # S matmul overwrites anyway
                            nc.tensor.matmul(
                                sAB[:, 0, :], dmask_sb[:, 0, :], xt_sb[0][:, 0:512],
                                start=True, stop=True, skip_group_check=True)
                        # S^T for s-block j against all remaining t-blocks in
                        # one matmul per head (the diagonal block's upper
                        # triangle is zeroed after exp, not PSUM-seeded)
                        HALVES = ((0, slice(0, 64)), (1, slice(64, 128)))
                        for h2, rows in HALVES:
                            nc.tensor.matmul(
                                sAB[:, h2, off:512],
                                kT_all[rows, j, p, :],
                                qT_all[rows, 4 * I + lo:4 * I + 4, p, :],
                                start=True, stop=True,
                                tile_position=(h2 * 64, 0),
                                skip_group_check=True)
                        eAB = exp_pool.tile([128, 2, 512], bf, name="eAB", tag="eAB")
                        bias = padb_sb[:, j:j + 1]
                        nc.scalar.activation(
                            out=eAB[:, :, off:512], in_=sAB[:, :, off:512],
                            func=EXP, bias=bias, scale=0.125)
                        if jl >= 0:
                            # zero the diagonal block above the diagonal
                            # (both heads in one 2-byte fast-mode multiply)
                            nc.vector.tensor_mul(
                                eAB[:, :, off:off + 128],
                                eAB[:, :, off:off + 128], dmask_sb)
                        if prev is not None:
                            emit_av(*prev)
                        prev = (j, eAB)
                        it += 1
                        did_out = False
                        if pending_out:
                            pending_out.pop(0)()
                            did_out = True
                        dry = drip() == 0 and not did_out
                    emit_av(*prev)

                    # normalize + evacuate transposed attn straight into the
                    # out-projection's stationary layout: DVE copies free the
                    # PSUM accumulators fast, then ONE full-width
                    # reciprocal_approx_fast covers both heads (~0.7us vs
                    # 2x3.3us for the exact reciprocal; the custom DVE op
                    # requires base partition 0, hence the cpd staging); the
                    # normalize multiplies run lazily on idle GpSimd off the
                    # PE's critical path.
                    cpd = rc_pool.tile([128, 512], f32, name="cpd", tag="rcd")
                    cpns = []
                    for h2, psT in ((0, psTA), (1, psTB)):
                        half = slice(h2 * 64, h2 * 64 + 64)
                        cpn = rc_pool.tile([128, 512], f32, name="cpn", tag="rcn")
                        nc.vector.tensor_copy(out=cpn[half, :], in_=psT[0:D, :])
                        nc.vector.tensor_copy(out=cpd[half, :], in_=psT[D:128, :])
                        cpns.append((half, cpn))
                    rc = rc_pool.tile([128, 512], f32, name="rc", tag="rc")
                    nc.vector.reciprocal_approx_fast(rc, cpd)
                    for half, cpn in cpns:
                        def make_norm(cpn=cpn, rc=rc, aT=aT_I, p=p, half=half):
                            def emit():
                                nc.gpsimd.tensor_mul(
                                    aT[half, p, :], cpn[half, :], rc[half, :])
                            return emit
                        pending_out.append(make_norm())

                # out-projection units are deferred into the next window's
                # loop as more PE gap-filler
                def make_out_unit(aT, i, il, n):
                    def emit():
                        pso = psum.tile([128, 512], f32, name="pso", tag="sAB", bufs=2)
                        for c in range(4):
                            nc.tensor.matmul(
                                pso,
                                aT[:, c, il * 128:(il + 1) * 128],
                                wo_sb[c][:, n * 512:(n + 1) * 512],
                                start=(c == 0), stop=(c == 3))
                        dst = outp[i * 128:(i + 1) * 128, n * 512:(n + 1) * 512]
                        osb = out_pool.tile([128, 512], f32, name="osb", tag="osb")
                        nc.vector.tensor_copy(out=osb, in_=pso)
                        nc.gpsimd.dma_start(out=dst, in_=osb)
                    return emit
                for il in range(4):
                    for n in range(2):
                        pending_out.append(make_out_unit(aT_I, 4 * I + il, il, n))
                for _ in nxt:
                    pass
            for f in pending_out:
                f()

    nc.compile()
    return nc


def _get_program():
    global _PROG
    if _PROG is None:
        _PROG = _build_program()
    return _PROG


def _rope_tables():
    bf16 = ml_dtypes.bfloat16
    inv = 1.0 / (ROPE_BASE ** (np.arange(0, D, 2, dtype=np.float64) / D))
    f = np.arange(T, dtype=np.float64)[:, None] * inv[None, :]  # [T, 32]
    c = np.cos(f)
    s = np.sin(f)
    # both 32-col halves carry the same table value
    cosc = np.concatenate([c, c], axis=1).astype(bf16)  # [T, 64]
    sinc = np.concatenate([s, s], axis=1).astype(bf16)
    return cosc, sinc


def kernel(x, attention_mask, W_qkv, W_out, b_out):
    global _LAST_RESULTS
    from concourse.bass_utils import run_bass_kernel_spmd

    nc = _get_program()
    bf16 = ml_dtypes.bfloat16
    x = np.asarray(x, dtype=np.float32)
    attention_mask = np.asarray(attention_mask)
    W_qkv = np.asarray(W_qkv, dtype=np.float32)
    W_out = np.asarray(W_out, dtype=np.float32)
    b_out = np.asarray(b_out, dtype=np.float32)

    cosc, sinc = _rope_tables()
    tri01 = (np.arange(128)[:, None] <= np.arange(128)[None, :]).astype(np.float32)
    dmask = np.concatenate([tri01, tri01], axis=1).astype(bf16)  # [128, 2*128]

    in_maps = []
    for g in range(NCORES):
        b, hg = g // 2, g % 2
        sl = slice(hg * 512, hg * 512 + 512)
        wq = W_qkv[:, 0 * C:][:, sl]
        wk = W_qkv[:, 1 * C:2 * C][:, sl]
        wv = W_qkv[:, 2 * C:3 * C][:, sl]
        wqkv_g = np.ascontiguousarray(
            np.concatenate([wq, wk, wv], axis=1)).astype(bf16)
        xT_g = np.ascontiguousarray(x[b].T).astype(bf16)
        wout_g = np.ascontiguousarray(W_out[sl, :]).astype(bf16)
        padb_g = np.ascontiguousarray(
            np.where(attention_mask[b] != 0, 0.0, -1e30)
            .astype(np.float32).reshape(TB, 128).T)
        in_maps.append({
            "xT": xT_g, "wqkv": wqkv_g, "wout": wout_g,
            "cosc": cosc, "sinc": sinc, "padb": padb_g, "dmask": dmask,
        })

    res = run_bass_kernel_spmd(nc, in_maps, list(range(NCORES)))
    _LAST_RESULTS = res
    out = np.empty((B, T, C), dtype=np.float32)
    for b in range(B):
        out[b] = res.results[2 * b]["outp"] + res.results[2 * b + 1]["outp"] + b_out
    return out
